# revision 1
# baseline (speedup 1.0000x reference)
"""Trainium2 Bass kernel for CorrelatedSphericalField sampling (bf16 v4).

Math (validated against the jax reference):
  coeffs[t] = PHI^t * d_t,   d_t = d_{t-1} + PHI^{-t} * sigma_n (.) xi_{t-1},  d_0 = coeff0
  xs[t,n,k,m] = sum_l d[t,n,l,m] * pct[m,l,k]          (per-m Legendre GEMM)
  out[t,n,k,j] = 4pi * PHI^t * irfft_j(xs), as half-spectrum GEMMs:
      A[.., j] = sum_m xs_re[.., m] C[m, j],  B[.., j] = sum_m xs_im[.., m] S[m, j]
      out[.., 0:362] = A + B ;  out[.., 362+jj] = (A - B)[.., 360-jj]
  PHI^t and 4pi are folded into per-core C/S constants.

Distribution (8 cores, single launch):
  stages A+B sharded over m (46 of 368 zero-padded m's per core, all (t,n)),
  processed in 4 m-groups (16/12/10/8) pipelined with a chunked AllToAll of
  xs (shard dim = t); stage D sharded over t (core c handles t=c).

Data is bf16 end to end (fp32 PSUM accumulation, fp32 output).
"""
import numpy as np
import ml_dtypes

import concourse.bass as bass
import concourse.mybir as mybir
import concourse.tile as tile
from concourse.bass_utils import run_bass_kernel_spmd

# ---- problem constants (hardcoded; kernel must be self-contained) ----
T = 8
N = 16
L = 361          # number of degrees l (contraction dim of stage B)
L2 = 384         # L zero-padded to 3*128
KLAT = 361       # number of latitudes
M = 362          # number of orders m
NLON = 722
JH = 362         # half-spectrum output columns of stage D
NC = 8
MPAD = 368       # M padded to a multiple of NC
MC = MPAD // NC  # 46 m's per core
TN = T * N       # 128
E = 2
MEN = MC * E * N  # 1472

PHI = float(np.exp(-6.0 / 48.0))
FOUR_PI = float(4.0 * np.pi)

LCH = [(0, 128), (128, 256), (256, 384)]
MCH = [(0, 128), (128, 256), (256, 368)]
KCH = [(0, 128), (128, 256), (256, 361)]
# A2A m-chunks within a core (each AllToAll has a ~30us ncfw floor, so few)
MGRP = [(0, 24), (24, 46)]
G = len(MGRP)
# sigma' packed offsets: per group block [T, me_g] at SIG_OFF[g]
SIG_OFF = [T * E * ga for (ga, gb) in MGRP]

F32 = mybir.dt.float32
BF16 = mybir.dt.bfloat16
NPBF = ml_dtypes.bfloat16


def _dseg(a, b):
    """Split global-m [a,b) into (core, group, local-in-group m0, count, poff)
    segments at core (MC) and A2A-group boundaries."""
    segs = []
    mg = a
    while mg < b:
        c = mg // MC
        local = mg - c * MC
        g = next(i for i, (ga, gb) in enumerate(MGRP) if ga <= local < gb)
        hi = min(b, c * MC + MGRP[g][1])
        segs.append((c, g, local - MGRP[g][0], hi - mg, mg - a))
        mg = hi
    return segs


def _split_multi_waits(nc, max_inline=1):
    """The walrus build in this env accepts only one inline sync-wait per
    instruction; hoist extras onto same-engine NoOps placed just before."""
    ctr = 0
    for f in nc.m.functions:
        for bb in f.blocks:
            new = []
            for inst in bb.instructions:
                si = inst.sync_info
                if si is not None and si.on_wait and len(si.on_wait) > max_inline:
                    waits = list(si.on_wait)
                    keep = waits[-max_inline:]
                    for w in waits[:-max_inline]:
                        ctr += 1
                        nop = mybir.InstNoOp(name=f"I-wsplit-{ctr}",
                                             engine=inst.engine)
                        nop.sync_info = mybir.SyncInfo(on_wait=[w], on_update=[])
                        new.append(nop)
                    inst.sync_info = mybir.SyncInfo(
                        on_wait=keep, on_update=list(si.on_update))
                new.append(inst)
            bb.instructions = new


def build_nc(split_waits=True):
    nc = bass.Bass(num_devices=NC)

    # host layouts: xi [l, t, (m, e, n)], c0 [l, (m, e, n)],
    # sigp [l, group-packed (t, m_g, e)], pct [m, l(384), k]
    xi_p = nc.declare_dram_parameter("xi_t", [L2, T, MEN], BF16, isOutput=False)
    c0_p = nc.declare_dram_parameter("c0_t", [L2, MEN], BF16, isOutput=False)
    sig_p = nc.declare_dram_parameter("sigp", [L2, T * MC * E], BF16, isOutput=False)
    pct_p = nc.declare_dram_parameter("pct_t", [MC, L2, KLAT], BF16, isOutput=False)
    csC_p = nc.declare_dram_parameter("csC", [MPAD, JH], BF16, isOutput=False)
    csS_p = nc.declare_dram_parameter("csS", [MPAD, JH], BF16, isOutput=False)
    out_p = nc.declare_dram_parameter("out_t", [N, KLAT, NLON], F32, isOutput=True)

    with tile.TileContext(nc) as tc:
        with tc.tile_pool(name="dram", bufs=1, space="DRAM") as pdram:
            sends, recvs = [], []
            for g, (ga, gb) in enumerate(MGRP):
                mg = gb - ga
                sends.append(pdram.tile([TN, E, mg, KLAT], BF16,
                                        name=f"send{g}", tag=f"send{g}"))
                recvs.append(pdram.tile([TN, E, mg, KLAT], BF16,
                                        name=f"recv{g}", tag=f"recv{g}"))

            with (
                tc.tile_pool(name="per", bufs=1) as pa,
                tc.tile_pool(name="cs", bufs=1) as pcs,
                tc.tile_pool(name="xr", bufs=1) as pxr,
                tc.tile_pool(name="xi", bufs=2) as px,
                tc.tile_pool(name="w", bufs=4) as pw,
                tc.tile_pool(name="xs", bufs=4) as pxs,
                tc.tile_pool(name="psB", bufs=3, space="PSUM") as pp,
            ):
                # stage-D constants loaded up front (SP stream is in-order)
                csC_t, csS_t = [], []
                for mc, (ma, mb) in enumerate(MCH):
                    mp = mb - ma
                    ct = pcs.tile([mp, JH], BF16, name=f"csC{mc}", tag=f"csC{mc}")
                    st = pcs.tile([mp, JH], BF16, name=f"csS{mc}", tag=f"csS{mc}")
                    nc.sync.dma_start(ct[:], csC_p[ma:mb])
                    nc.sync.dma_start(st[:], csS_p[ma:mb])
                    csC_t.append(ct)
                    csS_t.append(st)
                xr = {}
                for e in range(E):
                    for mc, (ma, mb) in enumerate(MCH):
                        xr[(e, mc)] = pxr.tile([mb - ma, N * KLAT], BF16,
                                               name=f"xr{e}{mc}", tag=f"xr{e}{mc}")

                # persistent: sigma', per-(lc,g) d tiles
                sig_tiles = []
                for lc, (la, lb) in enumerate(LCH):
                    st_ = pa.tile([128, T * MC * E], BF16, tag=f"sig{lc}")
                    nc.sync.dma_start(st_[:], sig_p[la:lb])
                    sig_tiles.append(st_)
                d_tiles = {}
                for g, (ga, gb) in enumerate(MGRP):
                    me_g = (gb - ga) * E
                    for lc in range(3):
                        d_tiles[(lc, g)] = pa.tile([128, me_g, T, N], BF16,
                                                   name=f"d{lc}g{g}",
                                                   tag=f"d{lc}g{g}")

                for g, (ga, gb) in enumerate(MGRP):
                    sz = gb - ga
                    me_g = sz * E
                    men_g = me_g * N
                    # ---- stage A for group g -------------------------------
                    for lc, (la, lb) in enumerate(LCH):
                        dt_ = d_tiles[(lc, g)]
                        xi_sb = px.tile([128, T * men_g], BF16, tag="xi")
                        c0_sb = px.tile([128, men_g], BF16, tag="c0s")
                        nc.sync.dma_start(
                            xi_sb[:].rearrange("p (t q) -> p t q", t=T),
                            xi_p[la:lb, :, ga * E * N:gb * E * N])
                        nc.sync.dma_start(
                            c0_sb[:], c0_p[la:lb, ga * E * N:gb * E * N])
                        sig_b = sig_tiles[lc][
                            :, SIG_OFF[g]:SIG_OFF[g] + T * me_g][
                            :, :, None].broadcast_to([128, T * me_g, N])
                        # z = sigma' (.) xi, computed in place in xi_sb
                        z_v = xi_sb[:].rearrange("p (tq n) -> p tq n", n=N)
                        nc.vector.tensor_tensor(
                            out=z_v, in0=z_v, in1=sig_b,
                            op=mybir.AluOpType.mult)
                        nc.vector.tensor_copy(
                            dt_[:, :, 0, :],
                            c0_sb[:].rearrange("p (q n) -> p q n", n=N))
                        for t in range(1, T):
                            nc.vector.tensor_tensor(
                                out=dt_[:, :, t, :],
                                in0=dt_[:, :, t - 1, :],
                                in1=z_v[:, (t - 1) * me_g:t * me_g, :],
                                op=mybir.AluOpType.add)

                    # ---- stage B for group g -------------------------------
                    for mp_ in range(ga, gb, 2):
                        w = pw.tile([128, 2, 3, KLAT], BF16, tag="pct")
                        nc.sync.dma_start(
                            w[:],
                            pct_p[mp_:mp_ + 2].rearrange(
                                "m (c p) k -> p m c k", p=128))
                        for mi in range(2):
                            m = mp_ + mi
                            gm = m - ga
                            xs_sb = pxs.tile([TN, E, KLAT], BF16, tag="xsb")
                            for e in range(E):
                                ps = pp.tile([TN, KLAT], F32, tag=f"ps{e}")
                                for lc in range(3):
                                    nc.tensor.matmul(
                                        ps[:],
                                        d_tiles[(lc, g)][:, gm * E + e],
                                        w[:, mi, lc],
                                        start=(lc == 0), stop=(lc == 2))
                                if e == 0:
                                    nc.scalar.copy(xs_sb[:, 0], ps[:])
                                else:
                                    nc.vector.tensor_copy(xs_sb[:, 1], ps[:])
                            nc.sync.dma_start(sends[g][:, :, gm], xs_sb[:])

                    nc.gpsimd.collective_compute(
                        "AllToAll", mybir.AluOpType.bypass,
                        replica_groups=[list(range(NC))],
                        ins=[sends[g].opt()], outs=[recvs[g].opt()])

                # xs-recv loads, group-ordered, issued from the Vector queue
                # so they don't head-of-line-block the (in-order) SP stream.
                for g in range(G):
                    for e in range(E):
                        for mc, (ma, mb) in enumerate(MCH):
                            for (c, sg, mlg, cnt, poff) in _dseg(ma, mb):
                                if sg != g:
                                    continue
                                src = recvs[g][16 * c:16 * (c + 1), e,
                                               mlg:mlg + cnt, :]
                                nc.scalar.dma_start(
                                    xr[(e, mc)][poff:poff + cnt].rearrange(
                                        "p (n k) -> p n k", n=N),
                                    src.transpose([1, 0, 2]))

            # ---------------- stage D: iFFT GEMM over m ---------------------
            with (
                tc.tile_pool(name="o", bufs=6) as po,
                tc.tile_pool(name="psD", bufs=3, space="PSUM") as pp2,
            ):
                for n in range(N):
                    for (ka, kb) in KCH:
                        kp = kb - ka
                        psA = pp2.tile([kp, JH], F32, tag="psA")
                        psB = pp2.tile([kp, JH], F32, tag="psB")
                        for mc in range(3):
                            nc.tensor.matmul(
                                psA[:],
                                xr[(0, mc)][:, n * KLAT + ka:n * KLAT + kb],
                                csC_t[mc][:],
                                start=(mc == 0), stop=(mc == 2))
                        for mc in range(3):
                            nc.tensor.matmul(
                                psB[:],
                                xr[(1, mc)][:, n * KLAT + ka:n * KLAT + kb],
                                csS_t[mc][:],
                                start=(mc == 0), stop=(mc == 2))
                        a_sb = po.tile([kp, JH], F32, tag="a_sb")
                        b_sb = po.tile([kp, JH], F32, tag="b_sb")
                        oo = po.tile([kp, NLON], F32, tag="oo")
                        nc.scalar.copy(a_sb[:], psA[:])
                        nc.scalar.copy(b_sb[:], psB[:])
                        nc.vector.tensor_tensor(
                            out=oo[:, 0:JH], in0=a_sb[:], in1=b_sb[:],
                            op=mybir.AluOpType.add)
                        nc.vector.tensor_tensor(
                            out=oo[:, JH:NLON], in0=a_sb[:, JH - 2:0:-1],
                            in1=b_sb[:, JH - 2:0:-1],
                            op=mybir.AluOpType.subtract)
                        nc.sync.dma_start(out_p[n, ka:kb], oo[:])

    if split_waits:
        _split_multi_waits(nc)
    return nc


def prep_inputs(x, sigma_n, coeff0, xi, pct):
    """Host-side shard/stage: slice + transpose per-core inputs, build constants."""
    sigma_n = np.asarray(sigma_n, np.float32)
    coeff0 = np.asarray(coeff0, np.float32)
    xi = np.asarray(xi, np.float32)
    pct = np.asarray(pct, np.float32)

    padm = MPAD - M
    padl = L2 - L
    sig_pad = np.pad(sigma_n, ((0, padl), (0, padm)))
    c0_pad = np.pad(coeff0, ((0, 0), (0, padl), (0, padm), (0, 0)))
    xi_pad = np.pad(xi, ((0, 0), (0, 0), (0, padl), (0, padm), (0, 0)))
    pct_pad = np.pad(pct, ((0, padm), (0, padl), (0, 0)))

    # half-spectrum irfft matrices (fp64 host build)
    j = np.arange(JH, dtype=np.float64)
    mm = np.arange(M, dtype=np.float64)
    ang = 2.0 * np.pi * np.outer(mm, j) / NLON
    Cm = 2.0 * np.cos(ang)
    Cm[0, :] = 1.0
    Cm[M - 1, :] = np.cos(np.pi * j)
    Sm = -2.0 * np.sin(ang)
    Sm[0, :] = 0.0
    Sm[M - 1, :] = 0.0
    Cp = np.pad(Cm, ((0, padm), (0, 0)))
    Sp = np.pad(Sm, ((0, padm), (0, 0)))

    phi_inv = PHI ** -(np.arange(T, dtype=np.float64) + 1.0)

    in_maps = []
    for c in range(NC):
        msl = slice(c * MC, (c + 1) * MC)
        # [t,n,l,m,e] -> [l,t,m,e,n]
        xi_c = np.ascontiguousarray(
            np.transpose(xi_pad[:, :, :, msl, :], (2, 0, 3, 4, 1))
        ).reshape(L2, T, MEN).astype(NPBF)
        # [n,l,m,e] -> [l,m,e,n]
        c0_c = np.ascontiguousarray(
            np.transpose(c0_pad[:, :, msl, :], (1, 2, 3, 0))
        ).reshape(L2, MEN).astype(NPBF)
        # sigma' group-packed: per group block [t, m_g, e]
        sig_me = (sig_pad[:, None, msl] * phi_inv[None, :, None])  # [l, t, m]
        sig_me = np.repeat(sig_me[:, :, :, None], E, axis=3)       # [l, t, m, e]
        blocks = [np.ascontiguousarray(sig_me[:, :, ga:gb]).reshape(L2, -1)
                  for (ga, gb) in MGRP]
        sig_c = np.concatenate(blocks, axis=1)
        pct_c = np.ascontiguousarray(pct_pad[msl]).astype(NPBF)
        scale = FOUR_PI * PHI ** c
        in_maps.append({
            "xi_t": xi_c,
            "c0_t": c0_c,
            "sigp": np.ascontiguousarray(sig_c).astype(NPBF),
            "pct_t": pct_c,
            "csC": (scale * Cp).astype(NPBF),
            "csS": (scale * Sp).astype(NPBF),
        })
    return in_maps


_NC_CACHE = None


def kernel(x, sigma_n, coeff0, xi, pct):
    global _NC_CACHE
    in_maps = prep_inputs(x, sigma_n, coeff0, xi, pct)
    if _NC_CACHE is None:
        _NC_CACHE = build_nc()
    res = run_bass_kernel_spmd(_NC_CACHE, in_maps, list(range(NC)))
    out = np.stack([res.results[c]["out_t"] for c in range(NC)], axis=0)
    return out.reshape(T, 1, 1, N, KLAT, NLON)



# revision 8
# speedup vs baseline: 1.1621x; 1.1621x over previous
"""Trainium2 Bass kernel for CorrelatedSphericalField sampling (fp16 v5).

Math (validated against the jax reference):
  coeffs[t] = PHI^t * d_t,   d_t = d_{t-1} + PHI^{-t} * sigma_n (.) xi_{t-1},  d_0 = coeff0
  xs[t,n,k,m] = sum_l d[t,n,l,m] * pct[m,l,k]          (per-m Legendre GEMM)
  out[t,n,k,j] = 4pi * PHI^t * irfft_j(xs), as half-spectrum GEMMs:
      A[.., j] = sum_m xs_re[.., m] C[m, j],  B[.., j] = sum_m xs_im[.., m] S[m, j]
      out[.., 0:362] = A + B ;  out[.., 362+jj] = (A - B)[.., 360-jj]
  PHI^t and 4pi are folded into per-core C/S constants.

Distribution (8 cores, single launch):
  m's are INTERLEAVED: core c owns global m = 8*i + c for i in [0,46).
  Since d[l,m] = 0 for l < m (sigma_n lower-triangular), l-chunks with
  lb <= 8*i are skipped: group 0 (i 0..16) uses l-chunks {0,1,2},
  group 1 (i 16..32) uses {1,2}, group 2 (i 32..46) uses {2}.
  Stages A+B run per m-group; each group is shipped with its own AllToAll
  (shard dim = t) and becomes exactly one 128-partition (112 for g2)
  contraction chunk of the stage-D iFFT GEMM.  Stage D accumulates group
  partials into fp16 SBUF accumulators so phases 0/1 overlap the A2A
  stream; only A2A2 + phase 2 (2 matmuls + combine + store) is exposed.

Data is fp16 end to end (fp32 PSUM accumulation); out is fp16, cast to
fp32 on the host.
"""
import numpy as np

import concourse.bass as bass
import concourse.mybir as mybir
import concourse.tile as tile
from concourse.bass_utils import run_bass_kernel_spmd

# ---- problem constants (hardcoded; kernel must be self-contained) ----
T = 8
N = 16
L = 361          # number of degrees l (contraction dim of stage B)
L2 = 384         # L zero-padded to 3*128
KLAT = 361       # number of latitudes
M = 362          # number of orders m
NLON = 722
JH = 362         # half-spectrum output columns of stage D
NC = 8
MC = 46          # m's per core (interleaved: m = 8*i + c)
TN = T * N       # 128
E = 2

PHI = float(np.exp(-6.0 / 48.0))
FOUR_PI = float(4.0 * np.pi)

LCH = [(0, 128), (128, 256), (256, 384)]
KCH = [(0, 128), (128, 256), (256, 361)]
# m-groups (local i ranges); group g needs l-chunks lc >= g
MGRP = [(0, 16), (16, 32), (32, 46)]
G = len(MGRP)
# l-chunks needed by group g (triangular skip: d[l,m]=0 for l<m, m_min=8i)
GLCS = [[0, 1, 2], [1, 2], [2]]
# w-quad splits per group (m-indices within the group)
def _quads(sz):
    if sz == 16:
        return [(0, 4), (4, 8), (8, 12), (12, 16)]
    return [(0, 4), (4, 8), (8, 11), (11, 14)]

F32 = mybir.dt.float32
F16 = mybir.dt.float16
NPF16 = np.float16


def _split_multi_waits(nc, max_inline=1):
    """The walrus build in this env accepts only one inline sync-wait per
    instruction; hoist extras onto same-engine NoOps placed just before."""
    ctr = 0
    for f in nc.m.functions:
        for bb in f.blocks:
            new = []
            for inst in bb.instructions:
                si = inst.sync_info
                if si is not None and si.on_wait and len(si.on_wait) > max_inline:
                    waits = list(si.on_wait)
                    keep = waits[-max_inline:]
                    for w in waits[:-max_inline]:
                        ctr += 1
                        nop = mybir.InstNoOp(name=f"I-wsplit-{ctr}",
                                             engine=inst.engine)
                        nop.sync_info = mybir.SyncInfo(on_wait=[w], on_update=[])
                        new.append(nop)
                    inst.sync_info = mybir.SyncInfo(
                        on_wait=keep, on_update=list(si.on_update))
                new.append(inst)
            bb.instructions = new


def build_nc(split_waits=True):
    nc = bass.Bass(num_devices=NC)

    # host layouts (per core, see prep_inputs):
    #  xi_g{g}_l{lc}:  [128(l), T, mg, E, N]   innovations, sigma'-scaled order
    #  c0_g{g}_l{lc}:  [128(l), mg, E, N]
    #  sig_g{g}_l{lc}: [128(l), T, mg, E]
    #  pct_g{g}_l{lc}: [128(l), mg, KLAT]
    #  csC_g{g}/csS_g{g}: [8*mg, JH]  (permuted rows to xr partition order,
    #                                  scaled by 4pi*PHI^me for my rank)
    xi_p, c0_p, sig_p, pct_p, csC_p, csS_p = {}, {}, {}, {}, {}, {}
    for g, (ga, gb) in enumerate(MGRP):
        mg = gb - ga
        for lc in GLCS[g]:
            xi_p[(g, lc)] = nc.declare_dram_parameter(
                f"xi_g{g}_l{lc}", [128, T * mg * E * N], F16, isOutput=False)
            c0_p[(g, lc)] = nc.declare_dram_parameter(
                f"c0_g{g}_l{lc}", [128, mg * E * N], F16, isOutput=False)
            sig_p[(g, lc)] = nc.declare_dram_parameter(
                f"sig_g{g}_l{lc}", [128, T * mg * E], F16, isOutput=False)
            pct_p[(g, lc)] = nc.declare_dram_parameter(
                f"pct_g{g}_l{lc}", [128, mg, KLAT], F16, isOutput=False)
        csC_p[g] = nc.declare_dram_parameter(
            f"csC_g{g}", [NC * mg, JH], F16, isOutput=False)
        csS_p[g] = nc.declare_dram_parameter(
            f"csS_g{g}", [NC * mg, JH], F16, isOutput=False)
    out_p = nc.declare_dram_parameter("out_t", [N, KLAT, NLON], F16,
                                      isOutput=True)

    with tile.TileContext(nc) as tc:
        with tc.tile_pool(name="dram", bufs=1, space="DRAM") as pdram:
            sends, recvs = [], []
            for g, (ga, gb) in enumerate(MGRP):
                mg = gb - ga
                sends.append(pdram.tile([TN, E, mg, KLAT], F16,
                                        name=f"send{g}", tag=f"send{g}"))
                recvs.append(pdram.tile([TN, E, mg, KLAT], F16,
                                        name=f"recv{g}", tag=f"recv{g}"))

            with (
                tc.tile_pool(name="cs", bufs=1) as pcs,
                tc.tile_pool(name="xr", bufs=2) as pxr,
            ):
                # stage-D constants (fp16, permuted+scaled on host)
                csC_t, csS_t, xr = [], [], {}
                for g, (ga, gb) in enumerate(MGRP):
                    mp = NC * (gb - ga)
                    ct = pcs.tile([mp, JH], F16, name=f"csC{g}", tag=f"csC{g}")
                    st = pcs.tile([mp, JH], F16, name=f"csS{g}", tag=f"csS{g}")
                    nc.sync.dma_start(ct[:], csC_p[g][:])
                    nc.sync.dma_start(st[:], csS_p[g][:])
                    csC_t.append(ct)
                    csS_t.append(st)

                # ---------- stages A + B per group, then AllToAll ----------
                with (
                    tc.tile_pool(name="sg", bufs=1) as psig,
                    tc.tile_pool(name="dd", bufs=2) as pd,
                    tc.tile_pool(name="xi", bufs=2) as px,
                    tc.tile_pool(name="w", bufs=2) as pw,
                    tc.tile_pool(name="xs", bufs=2) as pxs,
                    tc.tile_pool(name="psB", bufs=4, space="PSUM") as pp,
                ):
                    for g, (ga, gb) in enumerate(MGRP):
                        mg = gb - ga
                        me_g = mg * E
                        d_t = {}
                        # ---- stage A: AR(1) prefix d_t (vector) ----------
                        for lc in GLCS[g]:
                            sig_sb = psig.tile([128, T * me_g], F16,
                                               tag=f"sig{lc}")
                            nc.sync.dma_start(sig_sb[:], sig_p[(g, lc)][:])
                            xi_sb = px.tile([128, T * me_g * N], F16,
                                            tag=f"xi{lc}")
                            nc.sync.dma_start(xi_sb[:], xi_p[(g, lc)][:])
                            c0_sb = px.tile([128, me_g * N], F16,
                                            tag=f"c0{lc}")
                            nc.sync.dma_start(c0_sb[:], c0_p[(g, lc)][:])
                            dt_ = pd.tile([128, me_g, T, N], F16,
                                          tag=f"d{lc}")
                            d_t[lc] = dt_
                            sig_b = sig_sb[:].rearrange(
                                "p (t q) -> p t q", t=T)[
                                :, :, :, None].broadcast_to([128, T, me_g, N])
                            z_v = xi_sb[:].rearrange(
                                "p (t q n) -> p t q n", t=T, n=N)
                            nc.vector.tensor_tensor(
                                out=z_v, in0=z_v, in1=sig_b,
                                op=mybir.AluOpType.mult)
                            nc.vector.tensor_copy(
                                d_t[lc][:, :, 0, :],
                                c0_sb[:].rearrange("p (q n) -> p q n", n=N))
                            for t in range(1, T):
                                nc.vector.tensor_tensor(
                                    out=dt_[:, :, t, :],
                                    in0=dt_[:, :, t - 1, :],
                                    in1=z_v[:, t - 1],
                                    op=mybir.AluOpType.add)

                        # ---- stage B: per-m Legendre GEMM ----------------
                        lcs = GLCS[g]
                        for (qa, qb) in _quads(mg):
                            qm = qb - qa
                            wq = pw.tile([128, 4, len(lcs), KLAT], F16,
                                         tag="wq")
                            for li, lc in enumerate(lcs):
                                nc.sync.dma_start(
                                    wq[:, 0:qm, li],
                                    pct_p[(g, lc)][:, qa:qb])
                            xs_sb = pxs.tile([TN, E, 4, KLAT], F16, tag="xsb")
                            for mi in range(qm):
                                m = qa + mi
                                for e in range(E):
                                    ps = pp.tile([TN, KLAT], F32, tag="psB")
                                    for li, lc in enumerate(lcs):
                                        nc.tensor.matmul(
                                            ps[:],
                                            d_t[lc][:, m * E + e],
                                            wq[:, mi, li],
                                            start=(li == 0),
                                            stop=(li == len(lcs) - 1))
                                    if e == 0:
                                        nc.scalar.copy(xs_sb[:, 0, mi], ps[:])
                                    else:
                                        nc.vector.tensor_copy(
                                            xs_sb[:, 1, mi], ps[:])
                            nc.sync.dma_start(
                                sends[g][:, :, qa:qb], xs_sb[:, :, 0:qm])

                        nc.gpsimd.collective_compute(
                            "AllToAll", mybir.AluOpType.bypass,
                            replica_groups=[list(range(NC))],
                            ins=[sends[g].opt()], outs=[recvs[g].opt()])

                    # xs-recv loads, all at the END of the scalar stream so
                    # they never head-of-line-block stage-B drains; xr[(e,g)]
                    # partitions p = c*mg + i  <->  global m = 8*(ga+i) + c
                    for g, (ga, gb) in enumerate(MGRP):
                        mg = gb - ga
                        for e in range(E):
                            xrt = pxr.tile([NC * mg, N * KLAT], F16,
                                           name=f"xr{e}{g}", tag=f"xr{e}{g}",
                                           bufs=1)
                            xr[(e, g)] = xrt
                            for c in range(NC):
                                nc.scalar.dma_start(
                                    xrt[c * mg:(c + 1) * mg].rearrange(
                                        "p (n k) -> p n k", n=N),
                                    recvs[g][16 * c:16 * (c + 1), e]
                                    .transpose([1, 0, 2]))

                # ------- stage D: iFFT GEMM, 3 phases over m-groups -------
                with (
                    tc.tile_pool(name="acc", bufs=1) as pacc,
                    tc.tile_pool(name="o", bufs=4) as po,
                    tc.tile_pool(name="psD", bufs=4, space="PSUM") as pp2,
                ):
                    accA = pacc.tile([128, N * 3 * JH], F16, tag="accA")
                    accB = pacc.tile([128, N * 3 * JH], F16, tag="accB")
                    aA = accA[:].rearrange("p (q j) -> p q j", j=JH)
                    aB = accB[:].rearrange("p (q j) -> p q j", j=JH)

                    for g in range(G):
                        last = g == G - 1
                        for n in range(N):
                            for kc, (ka, kb) in enumerate(KCH):
                                kp = kb - ka
                                q = n * 3 + kc
                                psA = pp2.tile([kp, JH], F32, tag="psA")
                                psB = pp2.tile([kp, JH], F32, tag="psB")
                                nc.tensor.matmul(
                                    psA[:],
                                    xr[(0, g)][:, n * KLAT + ka:n * KLAT + kb],
                                    csC_t[g][:], start=True, stop=True)
                                nc.tensor.matmul(
                                    psB[:],
                                    xr[(1, g)][:, n * KLAT + ka:n * KLAT + kb],
                                    csS_t[g][:], start=True, stop=True)
                                if g == 0:
                                    nc.vector.tensor_copy(aA[0:kp, q], psA[:])
                                    nc.scalar.copy(aB[0:kp, q], psB[:])
                                elif not last:
                                    nc.vector.tensor_tensor(
                                        out=aA[0:kp, q], in0=aA[0:kp, q],
                                        in1=psA[:], op=mybir.AluOpType.add)
                                    nc.vector.tensor_tensor(
                                        out=aB[0:kp, q], in0=aB[0:kp, q],
                                        in1=psB[:], op=mybir.AluOpType.add)
                                else:
                                    t1 = po.tile([kp, JH], F16, tag="t1")
                                    t2 = po.tile([kp, JH], F16, tag="t2")
                                    oo = po.tile([kp, NLON], F16, tag="oo")
                                    nc.vector.tensor_tensor(
                                        out=t1[:], in0=aA[0:kp, q],
                                        in1=psA[:], op=mybir.AluOpType.add)
                                    nc.vector.tensor_tensor(
                                        out=t2[:], in0=aB[0:kp, q],
                                        in1=psB[:], op=mybir.AluOpType.add)
                                    nc.vector.tensor_tensor(
                                        out=oo[:, 0:JH], in0=t1[:], in1=t2[:],
                                        op=mybir.AluOpType.add)
                                    nc.gpsimd.tensor_tensor(
                                        out=oo[:, JH:NLON],
                                        in0=t1[:, JH - 2:0:-1],
                                        in1=t2[:, JH - 2:0:-1],
                                        op=mybir.AluOpType.subtract)
                                    nc.sync.dma_start(out_p[n, ka:kb], oo[:])

    if split_waits:
        _split_multi_waits(nc)
    return nc


def prep_inputs(x, sigma_n, coeff0, xi, pct):
    """Host-side shard/stage: interleaved m-assignment, triangular-skip
    blocks, per-(group, l-chunk) contiguous layouts, fp16."""
    sigma_n = np.asarray(sigma_n, np.float64)
    coeff0 = np.asarray(coeff0, np.float32)
    xi = np.asarray(xi, np.float32)
    pct = np.asarray(pct, np.float32)

    # zero-pad l to 384 and m to 8*MC = 368
    MP = NC * MC
    padl = L2 - L
    padm = MP - M
    sig_pad = np.pad(sigma_n, ((0, padl), (0, padm)))
    c0_pad = np.pad(coeff0, ((0, 0), (0, padl), (0, padm), (0, 0)))
    xi_pad = np.pad(xi, ((0, 0), (0, 0), (0, padl), (0, padm), (0, 0)))
    pct_pad = np.pad(pct, ((0, padm), (0, padl), (0, 0)))

    # half-spectrum irfft matrices (fp64 host build)
    j = np.arange(JH, dtype=np.float64)
    mm = np.arange(M, dtype=np.float64)
    ang = 2.0 * np.pi * np.outer(mm, j) / NLON
    Cm = 2.0 * np.cos(ang)
    Cm[0, :] = 1.0
    Cm[M - 1, :] = np.cos(np.pi * j)
    Sm = -2.0 * np.sin(ang)
    Sm[0, :] = 0.0
    Sm[M - 1, :] = 0.0
    Cp = np.pad(Cm, ((0, padm), (0, 0)))
    Sp = np.pad(Sm, ((0, padm), (0, 0)))

    phi_inv = PHI ** -(np.arange(T, dtype=np.float64) + 1.0)

    in_maps = []
    for c in range(NC):
        msel = 8 * np.arange(MC) + c          # global m's owned by core c
        dmap = {}
        for g, (ga, gb) in enumerate(MGRP):
            mg = gb - ga
            ms = msel[ga:gb]
            for lc in GLCS[g]:
                la, lb = LCH[lc]
                # xi [t,n,l,m,e] -> [l, t, m, e, n], sigma' folded later on
                # device; here just reorder
                xi_b = np.ascontiguousarray(np.transpose(
                    xi_pad[:, :, la:lb][:, :, :, ms, :],
                    (2, 0, 3, 4, 1))).reshape(128, -1).astype(NPF16)
                dmap[f"xi_g{g}_l{lc}"] = xi_b
                c0_b = np.ascontiguousarray(np.transpose(
                    c0_pad[:, la:lb][:, :, ms, :],
                    (1, 2, 3, 0))).reshape(128, -1).astype(NPF16)
                dmap[f"c0_g{g}_l{lc}"] = c0_b
                # sigma' [l, t, m, e]
                sg = sig_pad[la:lb][:, None, ms] * phi_inv[None, :, None]
                sg = np.repeat(sg[:, :, :, None], E, axis=3)
                dmap[f"sig_g{g}_l{lc}"] = np.ascontiguousarray(
                    sg).reshape(128, -1).astype(NPF16)
                # pct [m, l, k] -> [l, m, k]
                pc = np.ascontiguousarray(np.transpose(
                    pct_pad[ms, la:lb], (1, 0, 2))).astype(NPF16)
                dmap[f"pct_g{g}_l{lc}"] = pc
            # stage-D constants: xr partition p = cc*mg + i <-> m = 8*(ga+i)+cc
            rows = np.empty((NC * mg,), dtype=np.int64)
            for cc in range(NC):
                rows[cc * mg:(cc + 1) * mg] = 8 * (ga + np.arange(mg)) + cc
            scale = FOUR_PI * PHI ** c
            dmap[f"csC_g{g}"] = (scale * Cp[rows]).astype(NPF16)
            dmap[f"csS_g{g}"] = (scale * Sp[rows]).astype(NPF16)
        in_maps.append(dmap)
    return in_maps


_NC_CACHE = None


def kernel(x, sigma_n, coeff0, xi, pct):
    global _NC_CACHE
    in_maps = prep_inputs(x, sigma_n, coeff0, xi, pct)
    if _NC_CACHE is None:
        _NC_CACHE = build_nc()
    res = run_bass_kernel_spmd(_NC_CACHE, in_maps, list(range(NC)))
    out = np.stack([np.asarray(res.results[c]["out_t"], np.float32)
                    for c in range(NC)], axis=0)
    return out.reshape(T, 1, 1, N, KLAT, NLON)


# revision 19
# speedup vs baseline: 1.2099x; 1.0412x over previous
"""Trainium2 Bass kernel for CorrelatedSphericalField sampling (fp16 v5).

Math (validated against the jax reference):
  coeffs[t] = PHI^t * d_t,   d_t = d_{t-1} + PHI^{-t} * sigma_n (.) xi_{t-1},  d_0 = coeff0
  xs[t,n,k,m] = sum_l d[t,n,l,m] * pct[m,l,k]          (per-m Legendre GEMM)
  out[t,n,k,j] = 4pi * PHI^t * irfft_j(xs), as half-spectrum GEMMs:
      A[.., j] = sum_m xs_re[.., m] C[m, j],  B[.., j] = sum_m xs_im[.., m] S[m, j]
      out[.., 0:362] = A + B ;  out[.., 362+jj] = (A - B)[.., 360-jj]
  PHI^t and 4pi are folded into per-core C/S constants.

Distribution (8 cores, single launch):
  m's are INTERLEAVED: core c owns global m = 8*i + c for i in [0,46).
  Since d[l,m] = 0 for l < m (sigma_n lower-triangular), l-chunks with
  lb <= 8*i are skipped: group 0 (i 0..16) uses l-chunks {0,1,2},
  group 1 (i 16..32) uses {1,2}, group 2 (i 32..46) uses {2}.
  Stages A+B run per m-group; each group is shipped with its own AllToAll
  (shard dim = t) and becomes exactly one 128-partition (112 for g2)
  contraction chunk of the stage-D iFFT GEMM.  Stage D accumulates group
  partials into fp16 SBUF accumulators so phases 0/1 overlap the A2A
  stream; only A2A2 + phase 2 (2 matmuls + combine + store) is exposed.

Data is fp16 end to end (fp32 PSUM accumulation); out is fp16, cast to
fp32 on the host.
"""
import numpy as np

import concourse.bass as bass
import concourse.mybir as mybir
import concourse.tile as tile
from concourse.bass_utils import run_bass_kernel_spmd

# ---- problem constants (hardcoded; kernel must be self-contained) ----
T = 8
N = 16
L = 361          # number of degrees l (contraction dim of stage B)
L2 = 384         # L zero-padded to 3*128
KLAT = 361       # number of latitudes
M = 362          # number of orders m
NLON = 722
JH = 362         # half-spectrum output columns of stage D
NC = 8
MC = 46          # m's per core (interleaved: m = 8*i + c)
TN = T * N       # 128
E = 2

PHI = float(np.exp(-6.0 / 48.0))
FOUR_PI = float(4.0 * np.pi)

LCH = [(0, 128), (128, 256), (256, 384)]
KCH = [(0, 128), (128, 256), (256, 361)]
# m-groups (local i ranges); group g needs l-chunks lc >= g
MGRP = [(0, 16), (16, 32), (32, 46)]
G = len(MGRP)
# l-chunks needed by group g (triangular skip: d[l,m]=0 for l<m, m_min=8i)
GLCS = [[0, 1, 2], [1, 2], [2]]
# w-quad splits per group (m-indices within the group)
def _quads(sz):
    if sz == 16:
        return [(0, 4), (4, 8), (8, 12), (12, 16)]
    return [(0, 4), (4, 8), (8, 11), (11, 14)]

F32 = mybir.dt.float32
F16 = mybir.dt.float16
NPF16 = np.float16


def _split_multi_waits(nc, max_inline=1):
    """The walrus build in this env accepts only one inline sync-wait per
    instruction; hoist extras onto same-engine NoOps placed just before."""
    ctr = 0
    for f in nc.m.functions:
        for bb in f.blocks:
            new = []
            for inst in bb.instructions:
                si = inst.sync_info
                if si is not None and si.on_wait and len(si.on_wait) > max_inline:
                    waits = list(si.on_wait)
                    keep = waits[-max_inline:]
                    for w in waits[:-max_inline]:
                        ctr += 1
                        nop = mybir.InstNoOp(name=f"I-wsplit-{ctr}",
                                             engine=inst.engine)
                        nop.sync_info = mybir.SyncInfo(on_wait=[w], on_update=[])
                        new.append(nop)
                    inst.sync_info = mybir.SyncInfo(
                        on_wait=keep, on_update=list(si.on_update))
                new.append(inst)
            bb.instructions = new


def build_nc(split_waits=True):
    nc = bass.Bass(num_devices=NC)

    # host layouts (per core, see prep_inputs):
    #  xi_g{g}_l{lc}:  [128(l), T, mg, E, N]   innovations, sigma'-scaled order
    #  c0_g{g}_l{lc}:  [128(l), mg, E, N]
    #  sig_g{g}_l{lc}: [128(l), T, mg, E]
    #  pct_g{g}_l{lc}: [128(l), mg, KLAT]
    #  csC_g{g}/csS_g{g}: [8*mg, JH]  (permuted rows to xr partition order,
    #                                  scaled by 4pi*PHI^me for my rank)
    xi_p, c0_p, pct_p, csC_p, csS_p = {}, {}, {}, {}, {}
    for g, (ga, gb) in enumerate(MGRP):
        mg = gb - ga
        for lc in GLCS[g]:
            xi_p[(g, lc)] = nc.declare_dram_parameter(
                f"xi_g{g}_l{lc}", [128, T * mg * E * N], F16, isOutput=False)
            c0_p[(g, lc)] = nc.declare_dram_parameter(
                f"c0_g{g}_l{lc}", [128, mg * E * N], F16, isOutput=False)
            pct_p[(g, lc)] = nc.declare_dram_parameter(
                f"pct_g{g}_l{lc}", [128, mg, KLAT], F16, isOutput=False)
        csC_p[g] = nc.declare_dram_parameter(
            f"csC_g{g}", [NC * mg, JH], F16, isOutput=False)
        csS_p[g] = nc.declare_dram_parameter(
            f"csS_g{g}", [NC * mg, JH], F16, isOutput=False)
    id_p = nc.declare_dram_parameter("ident", [128, 128], F16, isOutput=False)
    out_p = nc.declare_dram_parameter("out_t", [N, KLAT, NLON], F16,
                                      isOutput=True)

    with tile.TileContext(nc) as tc:
        with tc.tile_pool(name="dram", bufs=1, space="DRAM") as pdram:
            sends, recvs = [], []
            for g, (ga, gb) in enumerate(MGRP):
                mg = gb - ga
                sends.append(pdram.tile([TN, E, mg, KLAT], F16,
                                        name=f"send{g}", tag=f"send{g}"))
                recvs.append(pdram.tile([TN, E, mg, KLAT], F16,
                                        name=f"recv{g}", tag=f"recv{g}"))

            with (
                tc.tile_pool(name="cs", bufs=1) as pcs,
                tc.tile_pool(name="xr", bufs=2) as pxr,
            ):
                csC_t, csS_t, xr = [], [], {}

                # ---------- stages A + B per group, then AllToAll ----------
                with (
                    tc.tile_pool(name="dd", bufs=2) as pd,
                    tc.tile_pool(name="xi", bufs=2) as px,
                    tc.tile_pool(name="w", bufs=3) as pw,
                    tc.tile_pool(name="xs", bufs=2) as pxs,
                    tc.tile_pool(name="psB", bufs=4, space="PSUM") as pp,
                ):
                    for g, (ga, gb) in enumerate(MGRP):
                        mg = gb - ga
                        me_g = mg * E
                        d_t = {}
                        # ---- stage A: AR(1) prefix d_t (vector) ----------
                        # xi arrives pre-scaled by sigma'*PHI^-t (host)
                        for lc in GLCS[g]:
                            xi_sb = px.tile([128, T * me_g * N], F16,
                                            tag="xi")
                            nc.sync.dma_start(xi_sb[:], xi_p[(g, lc)][:])
                            c0_sb = px.tile([128, me_g * N], F16,
                                            tag="c0")
                            nc.sync.dma_start(c0_sb[:], c0_p[(g, lc)][:])
                            dt_ = pd.tile([128, me_g, T, N], F16,
                                          tag=f"d{lc}",
                                          bufs=1 if lc == 0 else 2)
                            d_t[lc] = dt_
                            z_v = xi_sb[:].rearrange(
                                "p (t q n) -> p t q n", t=T, n=N)
                            nc.vector.tensor_copy(
                                d_t[lc][:, :, 0, :],
                                c0_sb[:].rearrange("p (q n) -> p q n", n=N))
                            for t in range(1, T):
                                nc.vector.tensor_tensor(
                                    out=dt_[:, :, t, :],
                                    in0=dt_[:, :, t - 1, :],
                                    in1=z_v[:, t - 1],
                                    op=mybir.AluOpType.add)

                        # ---- stage B: per-m Legendre GEMM ----------------
                        lcs = GLCS[g]
                        for (qa, qb) in _quads(mg):
                            qm = qb - qa
                            wq = pw.tile([128, 4, len(lcs), KLAT], F16,
                                         tag="wq")
                            for li, lc in enumerate(lcs):
                                nc.sync.dma_start(
                                    wq[:, 0:qm, li],
                                    pct_p[(g, lc)][:, qa:qb])
                            xs_sb = pxs.tile([TN, E, 4, KLAT], F16, tag="xsb")
                            for mi in range(qm):
                                m = qa + mi
                                for e in range(E):
                                    ps = pp.tile([TN, KLAT], F32, tag="psB")
                                    for li, lc in enumerate(lcs):
                                        nc.tensor.matmul(
                                            ps[:],
                                            d_t[lc][:, m * E + e],
                                            wq[:, mi, li],
                                            start=(li == 0),
                                            stop=(li == len(lcs) - 1))
                                    if e == 0:
                                        nc.scalar.copy(xs_sb[:, 0, mi], ps[:])
                                    else:
                                        nc.vector.tensor_copy(
                                            xs_sb[:, 1, mi], ps[:])
                            nc.sync.dma_start(
                                sends[g][:, :, qa:qb], xs_sb[:, :, 0:qm])

                        nc.gpsimd.collective_compute(
                            "AllToAll", mybir.AluOpType.bypass,
                            replica_groups=[list(range(NC))],
                            ins=[sends[g].opt()], outs=[recvs[g].opt()])

                    # stage-D constants + identity, after stage-B loads on q1
                    ident = pcs.tile([128, 128], F16, tag="ident")
                    nc.sync.dma_start(ident[:], id_p[:])
                    for g, (ga, gb) in enumerate(MGRP):
                        mp = NC * (gb - ga)
                        ct = pcs.tile([mp, JH], F16, name=f"csC{g}",
                                      tag=f"csC{g}")
                        st = pcs.tile([mp, JH], F16, name=f"csS{g}",
                                      tag=f"csS{g}")
                        nc.sync.dma_start(ct[:], csC_p[g][:])
                        nc.sync.dma_start(st[:], csS_p[g][:])
                        csC_t.append(ct)
                        csS_t.append(st)

                    # xs-recv loads, at the END of both DMA streams so they
                    # never head-of-line-block stage-B work; xr[(e,g)]
                    # partitions p = c*mg + i  <->  global m = 8*(ga+i) + c
                    for g, (ga, gb) in enumerate(MGRP):
                        mg = gb - ga
                        for e in range(E):
                            xrt = pxr.tile([NC * mg, N * KLAT], F16,
                                           name=f"xr{e}{g}", tag=f"xr{e}{g}",
                                           bufs=1)
                            xr[(e, g)] = xrt
                            eng = nc.sync if e == 0 else nc.scalar
                            for c in range(NC):
                                eng.dma_start(
                                    xrt[c * mg:(c + 1) * mg].rearrange(
                                        "p (n k) -> p n k", n=N),
                                    recvs[g][16 * c:16 * (c + 1), e]
                                    .transpose([1, 0, 2]))

                # ------- stage D: iFFT GEMM, 3 phases over m-groups -------
                with (
                    tc.tile_pool(name="acc", bufs=1) as pacc,
                    tc.tile_pool(name="o", bufs=4) as po,
                    tc.tile_pool(name="psD", bufs=4, space="PSUM") as pp2,
                ):
                    accA = pacc.tile([128, N * 3 * JH], F16, tag="accA")
                    accB = pacc.tile([128, N * 3 * JH], F16, tag="accB")
                    aA = accA[:].rearrange("p (q j) -> p q j", j=JH)
                    aB = accB[:].rearrange("p (q j) -> p q j", j=JH)

                    for g in range(G):
                        last = g == G - 1
                        for n in range(N):
                            for kc, (ka, kb) in enumerate(KCH):
                                kp = kb - ka
                                q = n * 3 + kc
                                psA = pp2.tile([kp, JH], F32, tag="psA")
                                psB = pp2.tile([kp, JH], F32, tag="psB")
                                # fresh group-g partial; for g>0 fold the
                                # accumulator back in on the tensor engine
                                nc.tensor.matmul(
                                    psA[:],
                                    xr[(0, g)][:, n * KLAT + ka:n * KLAT + kb],
                                    csC_t[g][:], start=True, stop=(g == 0))
                                if g > 0:
                                    nc.tensor.matmul(
                                        psA[:], ident[0:kp, 0:kp],
                                        aA[0:kp, q], start=False, stop=True)
                                nc.tensor.matmul(
                                    psB[:],
                                    xr[(1, g)][:, n * KLAT + ka:n * KLAT + kb],
                                    csS_t[g][:], start=True, stop=(g == 0))
                                if g > 0:
                                    nc.tensor.matmul(
                                        psB[:], ident[0:kp, 0:kp],
                                        aB[0:kp, q], start=False, stop=True)
                                if not last:
                                    nc.vector.tensor_copy(aA[0:kp, q], psA[:])
                                    nc.scalar.copy(aB[0:kp, q], psB[:])
                                else:
                                    t1 = po.tile([kp, JH], F16, tag="t1")
                                    t2 = po.tile([kp, JH], F16, tag="t2")
                                    oo = po.tile([kp, NLON], F16, tag="oo")
                                    nc.scalar.copy(t1[:], psA[:])
                                    nc.vector.tensor_copy(t2[:], psB[:])
                                    nc.vector.tensor_tensor(
                                        out=oo[:, 0:JH], in0=t1[:], in1=t2[:],
                                        op=mybir.AluOpType.add)
                                    nc.vector.tensor_tensor(
                                        out=oo[:, JH:NLON],
                                        in0=t1[:, JH - 2:0:-1],
                                        in1=t2[:, JH - 2:0:-1],
                                        op=mybir.AluOpType.subtract)
                                    nc.sync.dma_start(out_p[n, ka:kb], oo[:])

    if split_waits:
        _split_multi_waits(nc)
    return nc


def prep_inputs(x, sigma_n, coeff0, xi, pct):
    """Host-side shard/stage: interleaved m-assignment, triangular-skip
    blocks, per-(group, l-chunk) contiguous layouts, fp16."""
    sigma_n = np.asarray(sigma_n, np.float64)
    coeff0 = np.asarray(coeff0, np.float32)
    xi = np.asarray(xi, np.float32)
    pct = np.asarray(pct, np.float32)

    # zero-pad l to 384 and m to 8*MC = 368
    MP = NC * MC
    padl = L2 - L
    padm = MP - M
    sig_pad = np.pad(sigma_n, ((0, padl), (0, padm)))
    c0_pad = np.pad(coeff0, ((0, 0), (0, padl), (0, padm), (0, 0)))
    xi_pad = np.pad(xi, ((0, 0), (0, 0), (0, padl), (0, padm), (0, 0)))
    pct_pad = np.pad(pct, ((0, padm), (0, padl), (0, 0)))

    # half-spectrum irfft matrices (fp64 host build)
    j = np.arange(JH, dtype=np.float64)
    mm = np.arange(M, dtype=np.float64)
    ang = 2.0 * np.pi * np.outer(mm, j) / NLON
    Cm = 2.0 * np.cos(ang)
    Cm[0, :] = 1.0
    Cm[M - 1, :] = np.cos(np.pi * j)
    Sm = -2.0 * np.sin(ang)
    Sm[0, :] = 0.0
    Sm[M - 1, :] = 0.0
    Cp = np.pad(Cm, ((0, padm), (0, 0)))
    Sp = np.pad(Sm, ((0, padm), (0, 0)))

    phi_inv = PHI ** -(np.arange(T, dtype=np.float64) + 1.0)

    in_maps = []
    for c in range(NC):
        msel = 8 * np.arange(MC) + c          # global m's owned by core c
        dmap = {}
        for g, (ga, gb) in enumerate(MGRP):
            mg = gb - ga
            ms = msel[ga:gb]
            for lc in GLCS[g]:
                la, lb = LCH[lc]
                # z = sigma' * PHI^-t * xi, [t,n,l,m,e] -> [l, t, m, e, n]
                sgf = sig_pad[la:lb][:, ms]          # [128, mg]
                zb = (xi_pad[:, :, la:lb][:, :, :, ms, :]
                      * sgf[None, None, :, :, None]
                      * phi_inv[:, None, None, None, None])
                xi_b = np.ascontiguousarray(np.transpose(
                    zb, (2, 0, 3, 4, 1))).reshape(128, -1).astype(NPF16)
                dmap[f"xi_g{g}_l{lc}"] = xi_b
                c0_b = np.ascontiguousarray(np.transpose(
                    c0_pad[:, la:lb][:, :, ms, :],
                    (1, 2, 3, 0))).reshape(128, -1).astype(NPF16)
                dmap[f"c0_g{g}_l{lc}"] = c0_b
                # pct [m, l, k] -> [l, m, k]
                pc = np.ascontiguousarray(np.transpose(
                    pct_pad[ms, la:lb], (1, 0, 2))).astype(NPF16)
                dmap[f"pct_g{g}_l{lc}"] = pc
            # stage-D constants: xr partition p = cc*mg + i <-> m = 8*(ga+i)+cc
            rows = np.empty((NC * mg,), dtype=np.int64)
            for cc in range(NC):
                rows[cc * mg:(cc + 1) * mg] = 8 * (ga + np.arange(mg)) + cc
            scale = FOUR_PI * PHI ** c
            dmap[f"csC_g{g}"] = (scale * Cp[rows]).astype(NPF16)
            dmap[f"csS_g{g}"] = (scale * Sp[rows]).astype(NPF16)
        dmap["ident"] = np.eye(128, dtype=NPF16)
        in_maps.append(dmap)
    return in_maps


_NC_CACHE = None


def kernel(x, sigma_n, coeff0, xi, pct):
    global _NC_CACHE
    in_maps = prep_inputs(x, sigma_n, coeff0, xi, pct)
    if _NC_CACHE is None:
        _NC_CACHE = build_nc()
    res = run_bass_kernel_spmd(_NC_CACHE, in_maps, list(range(NC)))
    out = np.stack([np.asarray(res.results[c]["out_t"], np.float32)
                    for c in range(NC)], axis=0)
    return out.reshape(T, 1, 1, N, KLAT, NLON)


# revision 25
# speedup vs baseline: 1.4188x; 1.1727x over previous
"""Trainium2 Bass kernel for CorrelatedSphericalField sampling (fp16 v5).

Math (validated against the jax reference):
  coeffs[t] = PHI^t * d_t,   d_t = d_{t-1} + PHI^{-t} * sigma_n (.) xi_{t-1},  d_0 = coeff0
  xs[t,n,k,m] = sum_l d[t,n,l,m] * pct[m,l,k]          (per-m Legendre GEMM)
  out[t,n,k,j] = 4pi * PHI^t * irfft_j(xs), as half-spectrum GEMMs:
      A[.., j] = sum_m xs_re[.., m] C[m, j],  B[.., j] = sum_m xs_im[.., m] S[m, j]
      out[.., 0:362] = A + B ;  out[.., 362+jj] = (A - B)[.., 360-jj]
  PHI^t and 4pi are folded into per-core C/S constants.

Distribution (8 cores, single launch):
  m's are INTERLEAVED: core c owns global m = 8*i + c for i in [0,46).
  Since d[l,m] = 0 for l < m (sigma_n lower-triangular), l-chunks with
  lb <= 8*i are skipped: group 0 (i 0..16) uses l-chunks {0,1,2},
  group 1 (i 16..32) uses {1,2}, group 2 (i 32..46) uses {2}.
  Stages A+B run per m-group; each group is shipped with its own AllToAll
  (shard dim = t) and becomes exactly one 128-partition (112 for g2)
  contraction chunk of the stage-D iFFT GEMM.  Stage D accumulates group
  partials into fp16 SBUF accumulators so phases 0/1 overlap the A2A
  stream; only A2A2 + phase 2 (2 matmuls + combine + store) is exposed.

Data is fp16 end to end (fp32 PSUM accumulation); out is fp16, cast to
fp32 on the host.
"""
import numpy as np

import concourse.bass as bass
import concourse.mybir as mybir
import concourse.tile as tile
from concourse.bass_utils import run_bass_kernel_spmd

# ---- problem constants (hardcoded; kernel must be self-contained) ----
T = 8
N = 16
L = 361          # number of degrees l (contraction dim of stage B)
L2 = 384         # L zero-padded to 3*128
KLAT = 361       # number of latitudes
M = 362          # number of orders m
NLON = 722
JH = 362         # half-spectrum output columns of stage D
NC = 8
MC = 46          # m's per core (interleaved: m = 8*i + c)
TN = T * N       # 128
E = 2

PHI = float(np.exp(-6.0 / 48.0))
FOUR_PI = float(4.0 * np.pi)

LCH = [(0, 128), (128, 256), (256, 384)]
KCH = [(0, 128), (128, 256), (256, 361)]
# m-groups (local i ranges); group g needs l-chunks lc >= g
MGRP = [(0, 16), (16, 32), (32, 46)]
G = len(MGRP)
# l-chunks needed by group g (triangular skip: d[l,m]=0 for l<m, m_min=8i)
GLCS = [[0, 1, 2], [1, 2], [2]]
# w-quad splits per group (m-indices within the group)
def _quads(sz):
    if sz == 16:
        return [(0, 4), (4, 8), (8, 12), (12, 16)]
    return [(0, 4), (4, 8), (8, 11), (11, 14)]

F32 = mybir.dt.float32
F16 = mybir.dt.float16
NPF16 = np.float16


def _split_multi_waits(nc, max_inline=1):
    """The walrus build in this env accepts only one inline sync-wait per
    instruction; hoist extras onto same-engine NoOps placed just before."""
    ctr = 0
    for f in nc.m.functions:
        for bb in f.blocks:
            new = []
            for inst in bb.instructions:
                si = inst.sync_info
                if si is not None and si.on_wait and len(si.on_wait) > max_inline:
                    waits = list(si.on_wait)
                    keep = waits[-max_inline:]
                    for w in waits[:-max_inline]:
                        ctr += 1
                        nop = mybir.InstNoOp(name=f"I-wsplit-{ctr}",
                                             engine=inst.engine)
                        nop.sync_info = mybir.SyncInfo(on_wait=[w], on_update=[])
                        new.append(nop)
                    inst.sync_info = mybir.SyncInfo(
                        on_wait=keep, on_update=list(si.on_update))
                new.append(inst)
            bb.instructions = new


def build_nc(split_waits=True):
    nc = bass.Bass(num_devices=NC)

    # host layouts (per core, see prep_inputs):
    #  xi_g{g}_l{lc}:  [128(l), T, mg, E, N]   innovations, sigma'-scaled order
    #  c0_g{g}_l{lc}:  [128(l), mg, E, N]
    #  sig_g{g}_l{lc}: [128(l), T, mg, E]
    #  pct_g{g}_l{lc}: [128(l), mg, KLAT]
    #  csC_g{g}/csS_g{g}: [8*mg, JH]  (permuted rows to xr partition order,
    #                                  scaled by 4pi*PHI^me for my rank)
    xi_p, c0_p, pct_p, csC_p, csS_p = {}, {}, {}, {}, {}
    for g, (ga, gb) in enumerate(MGRP):
        mg = gb - ga
        for lc in GLCS[g]:
            xi_p[(g, lc)] = nc.declare_dram_parameter(
                f"xi_g{g}_l{lc}", [128, T * mg * E * N], F16, isOutput=False)
            c0_p[(g, lc)] = nc.declare_dram_parameter(
                f"c0_g{g}_l{lc}", [128, mg * E * N], F16, isOutput=False)
            pct_p[(g, lc)] = nc.declare_dram_parameter(
                f"pct_g{g}_l{lc}", [128, mg, KLAT], F16, isOutput=False)
        csC_p[g] = nc.declare_dram_parameter(
            f"csC_g{g}", [NC * mg, JH], F16, isOutput=False)
        csS_p[g] = nc.declare_dram_parameter(
            f"csS_g{g}", [NC * mg, JH], F16, isOutput=False)
    id_p = nc.declare_dram_parameter("ident", [128, 128], F16, isOutput=False)
    out_p = nc.declare_dram_parameter("out_t", [N, KLAT, NLON], F16,
                                      isOutput=True)

    with tile.TileContext(nc) as tc:
        with tc.tile_pool(name="dram", bufs=1, space="DRAM") as pdram:
            sends, recvs = [], []
            for g, (ga, gb) in enumerate(MGRP):
                mg = gb - ga
                sends.append(pdram.tile([TN, mg, E, KLAT], F16,
                                        name=f"send{g}", tag=f"send{g}"))
                recvs.append(pdram.tile([TN, mg, E, KLAT], F16,
                                        name=f"recv{g}", tag=f"recv{g}"))

            with (
                tc.tile_pool(name="cs", bufs=1) as pcs,
                tc.tile_pool(name="xr", bufs=2) as pxr,
            ):
                csC_t, csS_t, xr = [], [], {}

                # ---------- stages A + B per group, then AllToAll ----------
                with (
                    tc.tile_pool(name="dd", bufs=2) as pd,
                    tc.tile_pool(name="xi", bufs=2) as px,
                    tc.tile_pool(name="w", bufs=3) as pw,
                    tc.tile_pool(name="xs", bufs=2) as pxs,
                    tc.tile_pool(name="psB", bufs=4, space="PSUM") as pp,
                ):
                    for g, (ga, gb) in enumerate(MGRP):
                        mg = gb - ga
                        me_g = mg * E
                        d_t = {}
                        # ---- stage A: AR(1) prefix d_t (vector) ----------
                        # xi arrives pre-scaled by sigma'*PHI^-t (host)
                        for lc in GLCS[g]:
                            xi_sb = px.tile([128, T * me_g * N], F16,
                                            tag="xi")
                            nc.sync.dma_start(xi_sb[:], xi_p[(g, lc)][:])
                            c0_sb = px.tile([128, me_g * N], F16,
                                            tag="c0")
                            nc.sync.dma_start(c0_sb[:], c0_p[(g, lc)][:])
                            dt_ = pd.tile([128, me_g, T, N], F16,
                                          tag=f"d{lc}",
                                          bufs=1 if lc == 0 else 2)
                            d_t[lc] = dt_
                            z_v = xi_sb[:].rearrange(
                                "p (t q n) -> p t q n", t=T, n=N)
                            nc.vector.tensor_copy(
                                d_t[lc][:, :, 0, :],
                                c0_sb[:].rearrange("p (q n) -> p q n", n=N))
                            for t in range(1, T):
                                nc.vector.tensor_tensor(
                                    out=dt_[:, :, t, :],
                                    in0=dt_[:, :, t - 1, :],
                                    in1=z_v[:, t - 1],
                                    op=mybir.AluOpType.add)

                        # ---- stage B: per-m Legendre GEMM ----------------
                        lcs = GLCS[g]
                        for (qa, qb) in _quads(mg):
                            qm = qb - qa
                            wq = pw.tile([128, 4, len(lcs), KLAT], F16,
                                         tag="wq")
                            for li, lc in enumerate(lcs):
                                nc.sync.dma_start(
                                    wq[:, 0:qm, li],
                                    pct_p[(g, lc)][:, qa:qb])
                            xs_sb = pxs.tile([TN, 4, E, KLAT], F16, tag="xsb")
                            for mi in range(qm):
                                m = qa + mi
                                for e in range(E):
                                    ps = pp.tile([TN, KLAT], F32, tag="psB")
                                    for li, lc in enumerate(lcs):
                                        nc.tensor.matmul(
                                            ps[:],
                                            d_t[lc][:, m * E + e],
                                            wq[:, mi, li],
                                            start=(li == 0),
                                            stop=(li == len(lcs) - 1))
                                    if e == 0:
                                        nc.scalar.copy(xs_sb[:, mi, 0], ps[:])
                                    else:
                                        nc.vector.tensor_copy(
                                            xs_sb[:, mi, 1], ps[:])
                            nc.sync.dma_start(
                                sends[g][:, qa:qb], xs_sb[:, 0:qm])

                        nc.gpsimd.collective_compute(
                            "AllToAll", mybir.AluOpType.bypass,
                            replica_groups=[list(range(NC))],
                            ins=[sends[g].opt()], outs=[recvs[g].opt()])

                    # stage-D constants + identity, after stage-B loads on q1
                    ident = pcs.tile([128, 128], F16, tag="ident")
                    nc.sync.dma_start(ident[:], id_p[:])
                    for g, (ga, gb) in enumerate(MGRP):
                        mp = NC * (gb - ga)
                        ct = pcs.tile([mp, JH], F16, name=f"csC{g}",
                                      tag=f"csC{g}")
                        st = pcs.tile([mp, JH], F16, name=f"csS{g}",
                                      tag=f"csS{g}")
                        nc.sync.dma_start(ct[:], csC_p[g][:])
                        nc.sync.dma_start(st[:], csS_p[g][:])
                        csC_t.append(ct)
                        csS_t.append(st)

                    # xs-recv loads on sync+gpsimd queues (scalar/vector stay
                    # free for phase drains); xr[g] partitions p = c*mg + i
                    # <-> global m = 8*(ga+i) + c, free dims [n, e, k]
                    for g, (ga, gb) in enumerate(MGRP):
                        mg = gb - ga
                        xrt = pxr.tile([NC * mg, N * E * KLAT], F16,
                                       name=f"xr{g}", tag=f"xr{g}", bufs=1)
                        xr[g] = xrt[:].rearrange(
                            "p (n e k) -> p n e k", n=N, e=E)
                        for c in range(NC):
                            eng = nc.sync if c % 2 == 0 else nc.gpsimd
                            eng.dma_start(
                                xr[g][c * mg:(c + 1) * mg],
                                recvs[g][16 * c:16 * (c + 1)]
                                .transpose([1, 0, 2, 3]))

                # ------- stage D: iFFT GEMM, 3 phases over m-groups -------
                with (
                    tc.tile_pool(name="acc", bufs=1) as pacc,
                    tc.tile_pool(name="o", bufs=4) as po,
                    tc.tile_pool(name="psD", bufs=4, space="PSUM") as pp2,
                ):
                    accA = pacc.tile([128, N * 3 * JH], F16, tag="accA")
                    accB = pacc.tile([128, N * 3 * JH], F16, tag="accB")
                    aA = accA[:].rearrange("p (q j) -> p q j", j=JH)
                    aB = accB[:].rearrange("p (q j) -> p q j", j=JH)

                    for g in range(G):
                        last = g == G - 1
                        for n in range(N):
                            for kc, (ka, kb) in enumerate(KCH):
                                kp = kb - ka
                                q = n * 3 + kc
                                psA = pp2.tile([kp, JH], F32, tag="psA")
                                psB = pp2.tile([kp, JH], F32, tag="psB")
                                # fresh group-g partial; for g>0 fold the
                                # accumulator back in on the tensor engine
                                nc.tensor.matmul(
                                    psA[:],
                                    xr[g][:, n, 0, ka:kb],
                                    csC_t[g][:], start=True, stop=(g == 0))
                                if g > 0:
                                    nc.tensor.matmul(
                                        psA[:], ident[0:kp, 0:kp],
                                        aA[0:kp, q], start=False, stop=True)
                                nc.tensor.matmul(
                                    psB[:],
                                    xr[g][:, n, 1, ka:kb],
                                    csS_t[g][:], start=True, stop=(g == 0))
                                if g > 0:
                                    nc.tensor.matmul(
                                        psB[:], ident[0:kp, 0:kp],
                                        aB[0:kp, q], start=False, stop=True)
                                if not last:
                                    nc.vector.tensor_copy(aA[0:kp, q], psA[:])
                                    nc.scalar.copy(aB[0:kp, q], psB[:])
                                else:
                                    t1 = po.tile([kp, JH], F16, tag="t1")
                                    t2 = po.tile([kp, JH], F16, tag="t2")
                                    oo = po.tile([kp, NLON], F16, tag="oo")
                                    nc.scalar.copy(t1[:], psA[:])
                                    nc.vector.tensor_copy(t2[:], psB[:])
                                    nc.vector.tensor_tensor(
                                        out=oo[:, 0:JH], in0=t1[:], in1=t2[:],
                                        op=mybir.AluOpType.add)
                                    nc.vector.tensor_tensor(
                                        out=oo[:, JH:NLON],
                                        in0=t1[:, JH - 2:0:-1],
                                        in1=t2[:, JH - 2:0:-1],
                                        op=mybir.AluOpType.subtract)
                                    nc.sync.dma_start(out_p[n, ka:kb], oo[:])

    if split_waits:
        _split_multi_waits(nc)
    return nc


def prep_inputs(x, sigma_n, coeff0, xi, pct):
    """Host-side shard/stage: interleaved m-assignment, triangular-skip
    blocks, per-(group, l-chunk) contiguous layouts, fp16."""
    sigma_n = np.asarray(sigma_n, np.float64)
    coeff0 = np.asarray(coeff0, np.float32)
    xi = np.asarray(xi, np.float32)
    pct = np.asarray(pct, np.float32)

    # zero-pad l to 384 and m to 8*MC = 368
    MP = NC * MC
    padl = L2 - L
    padm = MP - M
    sig_pad = np.pad(sigma_n, ((0, padl), (0, padm)))
    c0_pad = np.pad(coeff0, ((0, 0), (0, padl), (0, padm), (0, 0)))
    xi_pad = np.pad(xi, ((0, 0), (0, 0), (0, padl), (0, padm), (0, 0)))
    pct_pad = np.pad(pct, ((0, padm), (0, padl), (0, 0)))

    # half-spectrum irfft matrices (fp64 host build)
    j = np.arange(JH, dtype=np.float64)
    mm = np.arange(M, dtype=np.float64)
    ang = 2.0 * np.pi * np.outer(mm, j) / NLON
    Cm = 2.0 * np.cos(ang)
    Cm[0, :] = 1.0
    Cm[M - 1, :] = np.cos(np.pi * j)
    Sm = -2.0 * np.sin(ang)
    Sm[0, :] = 0.0
    Sm[M - 1, :] = 0.0
    Cp = np.pad(Cm, ((0, padm), (0, 0)))
    Sp = np.pad(Sm, ((0, padm), (0, 0)))

    phi_inv = PHI ** -(np.arange(T, dtype=np.float64) + 1.0)

    in_maps = []
    for c in range(NC):
        msel = 8 * np.arange(MC) + c          # global m's owned by core c
        dmap = {}
        for g, (ga, gb) in enumerate(MGRP):
            mg = gb - ga
            ms = msel[ga:gb]
            for lc in GLCS[g]:
                la, lb = LCH[lc]
                # z = sigma' * PHI^-t * xi, [t,n,l,m,e] -> [l, t, m, e, n]
                sgf = sig_pad[la:lb][:, ms]          # [128, mg]
                zb = (xi_pad[:, :, la:lb][:, :, :, ms, :]
                      * sgf[None, None, :, :, None]
                      * phi_inv[:, None, None, None, None])
                xi_b = np.ascontiguousarray(np.transpose(
                    zb, (2, 0, 3, 4, 1))).reshape(128, -1).astype(NPF16)
                dmap[f"xi_g{g}_l{lc}"] = xi_b
                c0_b = np.ascontiguousarray(np.transpose(
                    c0_pad[:, la:lb][:, :, ms, :],
                    (1, 2, 3, 0))).reshape(128, -1).astype(NPF16)
                dmap[f"c0_g{g}_l{lc}"] = c0_b
                # pct [m, l, k] -> [l, m, k]
                pc = np.ascontiguousarray(np.transpose(
                    pct_pad[ms, la:lb], (1, 0, 2))).astype(NPF16)
                dmap[f"pct_g{g}_l{lc}"] = pc
            # stage-D constants: xr partition p = cc*mg + i <-> m = 8*(ga+i)+cc
            rows = np.empty((NC * mg,), dtype=np.int64)
            for cc in range(NC):
                rows[cc * mg:(cc + 1) * mg] = 8 * (ga + np.arange(mg)) + cc
            scale = FOUR_PI * PHI ** c
            dmap[f"csC_g{g}"] = (scale * Cp[rows]).astype(NPF16)
            dmap[f"csS_g{g}"] = (scale * Sp[rows]).astype(NPF16)
        dmap["ident"] = np.eye(128, dtype=NPF16)
        in_maps.append(dmap)
    return in_maps


_NC_CACHE = None


def kernel(x, sigma_n, coeff0, xi, pct):
    global _NC_CACHE
    in_maps = prep_inputs(x, sigma_n, coeff0, xi, pct)
    if _NC_CACHE is None:
        _NC_CACHE = build_nc()
    res = run_bass_kernel_spmd(_NC_CACHE, in_maps, list(range(NC)))
    out = np.stack([np.asarray(res.results[c]["out_t"], np.float32)
                    for c in range(NC)], axis=0)
    return out.reshape(T, 1, 1, N, KLAT, NLON)
